# revision 13
# baseline (speedup 1.0000x reference)
"""Multi-head attention (B=2, S=2048, D=1024, H=16) on 8 Trainium2 NeuronCores.

Sharding: core c -> (batch b = c//4, head group g = c%4), i.e. data parallel on
batch and tensor parallel on heads (4 heads = 256 features per core) for the
QKV projections. Attention runs fully local per (batch, head-group). The
output projection is computed as a LOCAL partial product against the row-slice
Wo[g*256:(g+1)*256, :] (full 1024 output columns), and the 4 partials of a
group are combined with a ReduceScatter(add) per 256-token q-half-chunk whose
output IS the kernel's external output. This keeps every matmul free of
collective dependencies (the collective is a pure sink), unlike an
AllGather-then-project scheme where the projection matmuls stall the in-order
PE queue while the gather is in flight.

Pipelining: the PE queue is in-order, so any instruction emitted before its
producer finishes stalls everything behind it. The attention inner loop is
software-pipelined: attnV(kg) is emitted 1-3 iterations after exp(kg), so the
ScalarEngine's exp latency (~1.1us per tile) never blocks the PE. This also
keeps the PE continuously busy, which matters twice: idle gaps drop the PE
from its 2.4 GHz max p-state to 1.2 GHz (ramp-up needs ~3us of continuous
execution). Deferred work (later Q^T projection chunks, the previous chunk's
output-projection partials, per-head softmax normalization) is drained into
the gaps one step per inner iteration.

Startup DMA is spread over four engine queues (sync/scalar/vector/gpsimd,
~235 GB/s each) so the 14 MB of inputs land in ~9us instead of ~60us serial.

Math notes (exact, not approximations):
  - bk is dropped: adding bk shifts every score in a row by a constant, and
    softmax is invariant to row-constant shifts.
  - bv and bo are folded into a single host-side bias add: softmax rows sum
    to 1, so attn @ (1 bv^T) = bv broadcast, and (out + bv) @ Wo + bo =
    out @ Wo + (bv @ Wo + bo).
  - bq is added on-device in the Q^T projection epilogue (per-partition add).
  - softmax skips max-subtraction: scores are ~N(0,1) for this problem's
    input distribution (|s| < ~7), far from fp32/bf16 exp overflow.
  - an all-ones mask (this problem's spec) is an identity; if a mask with
    zeros is ever passed, a masked kernel variant is compiled instead
    (multiply exp(scores) by the 0/1 mask — identical to adding -1e9).

Compute is bf16 on the TensorEngine (fp32 PSUM accumulation), exp on the
ScalarEngine in fp32. Scores are computed transposed (S^T[k_tok, q]) so that
attn @ V needs no transposes and the softmax denominator is obtained free via
an extra ones-column appended to V.
"""

import numpy as np
import ml_dtypes

try:
    import concourse.bass as bass  # noqa: F401
except ImportError:  # fresh interpreter without the repo on sys.path
    import sys

    for p in ("/opt/trn_rl_repo", "/root/.axon_site/_ro/trn_rl_repo"):
        if p not in sys.path:
            sys.path.insert(0, p)
    import concourse.bass as bass  # noqa: F401

import concourse.tile as tile
from concourse import bacc, mybir
from concourse.bass_utils import run_bass_kernel_spmd

BF16 = ml_dtypes.bfloat16
B, S, D, H = 2, 2048, 1024, 16
DK = D // H            # 64
N_CORES = 8
GROUPS = [[0, 1, 2, 3], [4, 5, 6, 7]]
FLOC = D // 4          # 256 features (4 heads) per core
P = 128

# Flipped by the test harness to collect an NTFF profile; harmless if the
# profiling hook is unavailable (tracing is skipped with a warning).
TRACE = False
LAST = {}

_BUILD_CACHE = {}


def _pmajor(ap):
    """View a [A*128, N] DRAM tensor as [128, A, N] (partition-major)."""
    return ap.rearrange("(a p) n -> p a n", p=P)


def _build(s, use_mask):
    key = (s, use_mask)
    if key in _BUILD_CACHE:
        return _BUILD_CACHE[key]

    f32 = mybir.dt.float32
    bf16 = mybir.dt.bfloat16
    nkt = D // P               # 8 k-tiles over the model dim
    nst = s // P               # 16 seq tiles of 128
    qcw = s // 4               # 512: q-chunk width
    hw_ = qcw // 2             # 256: ReduceScatter half-chunk width
    nft = FLOC // P            # 2 feature tiles per core
    nch_n = s // 512           # 4 input chunks for the projections

    nc = bacc.Bacc("TRN2", target_bir_lowering=False, debug=False,
                   enable_asserts=True, num_devices=N_CORES)

    qT = nc.dram_tensor("qT", [D, s], bf16, kind="ExternalInput").ap()
    kT = nc.dram_tensor("kT", [D, s], bf16, kind="ExternalInput").ap()
    vT = nc.dram_tensor("vT", [D, s], bf16, kind="ExternalInput").ap()
    wq = nc.dram_tensor("wq", [D, FLOC], bf16, kind="ExternalInput").ap()
    wk = nc.dram_tensor("wk", [D, FLOC], bf16, kind="ExternalInput").ap()
    wv = nc.dram_tensor("wv", [D, FLOC], bf16, kind="ExternalInput").ap()
    # Row-slice of Wo: Wo[g*FLOC:(g+1)*FLOC, :] — this core's contraction rows
    wo = nc.dram_tensor("wo", [FLOC, D], bf16, kind="ExternalInput").ap()
    bqp = nc.dram_tensor("bqp", [FLOC, 1], f32, kind="ExternalInput").ap()
    if use_mask:
        maskT = nc.dram_tensor("maskT", [s, s], bf16, kind="ExternalInput").ap()

    # ReduceScatter in/out per q-half-chunk. rs_in holds this core's partial
    # out^T [D, hw_]; the RS sums the 4 partials of the group and hands rank g
    # rows [g*FLOC:(g+1)*FLOC] — the final (biasless) output, written straight
    # into the external outputs.
    rs_in = [nc.dram_tensor(f"rs_in{x}", [D, hw_], bf16).ap()
             for x in range(8)]
    rs_out = [nc.dram_tensor(f"rs_out{x}", [FLOC, hw_], bf16).ap()
              for x in range(8)]
    # outT[outd, q] — columns x*hw_.. filled by a dram->dram copy after RS x
    outT = nc.dram_tensor("outT", [FLOC, s], bf16, kind="ExternalOutput").ap()

    EXP = mybir.ActivationFunctionType.Exp

    with tile.TileContext(nc) as tc:
        with (
            tc.tile_pool(name="persist", bufs=1) as pp,
            tc.tile_pool(name="xq", bufs=2) as xq_pool,
            tc.tile_pool(name="xk", bufs=2) as xk_pool,
            tc.tile_pool(name="xv", bufs=1) as xv_pool,
            tc.tile_pool(name="exp", bufs=6) as exp_pool,
            tc.tile_pool(name="msk", bufs=4) as msk_pool,
            tc.tile_pool(name="small", bufs=4) as small_pool,
            tc.tile_pool(name="rsb", bufs=1) as rsb_pool,
            tc.tile_pool(name="ps_s", bufs=2, space="PSUM") as ps_s,
            tc.tile_pool(name="ps_acc", bufs=2, space="PSUM") as ps_acc,
            tc.tile_pool(name="ps_misc", bufs=2, space="PSUM") as ps_misc,
        ):
            # ---- weights / constants, spread across DMA queues -------------
            w_sb = {}

            def load_w(nm, src, a, eng):
                t = pp.tile([P, a * src.shape[1]], bf16, tag=nm, name=nm)
                eng.dma_start(t.rearrange("p (a n) -> p a n", a=a), _pmajor(src))
                w_sb[nm] = t

            load_w("wk", wk, nkt, nc.sync)
            load_w("wq", wq, nkt, nc.sync)
            bq_sb = pp.tile([P, nft], f32, tag="bq", name="bq")
            nc.sync.dma_start(
                bq_sb.rearrange("p (a n) -> p a n", a=nft), _pmajor(bqp))
            load_w("wv", wv, nkt, nc.gpsimd)
            load_w("wo", wo, nft, nc.gpsimd)   # [128, nft*D]

            wq_sl = lambda kt, f: w_sb["wq"][:, kt * FLOC + f * P: kt * FLOC + (f + 1) * P]
            wk_sl = lambda kt, f: w_sb["wk"][:, kt * FLOC + f * P: kt * FLOC + (f + 1) * P]
            wv_sl = lambda kt: w_sb["wv"][:, kt * FLOC:(kt + 1) * FLOC]
            wo_sl = lambda ft, od: w_sb["wo"][:, ft * D + od * P: ft * D + (od + 1) * P]

            ones_sb = pp.tile([1, DK], bf16, tag="ones", name="ones")
            nc.vector.memset(ones_sb[:], 1.0)

            QT_sb = [pp.tile([P, s], bf16, tag=f"qtsb{f}", name=f"qtsb{f}")
                     for f in range(nft)]
            KT_sb = [pp.tile([P, s], bf16, tag=f"ktsb{f}", name=f"ktsb{f}")
                     for f in range(nft)]
            AOT_sb = [pp.tile([P, s], bf16, tag=f"aot{f}", name=f"aot{f}")
                      for f in range(nft)]
            V_sb = [pp.tile([P, 4 * (DK + 1)], bf16, tag=f"vsb{tt}", name=f"vsb{tt}")
                    for tt in range(nst)]

            # ---- input DMAs: kT split across sync+scalar queues, vT chunks
            # on the gpsimd queue (idle until the first RS), qT c0 on vector --
            kx, vx = {}, {}

            def load_k(nch, eng):
                t = xk_pool.tile([P, nkt * 512], bf16, name="xk",
                                 tag=f"xk{nch}")
                eng.dma_start(t.rearrange("p (a n) -> p a n", a=nkt),
                              _pmajor(kT)[:, :, nch * 512:(nch + 1) * 512])
                kx[nch] = t

            load_k(0, nc.sync)
            load_k(1, nc.sync)
            load_k(2, nc.scalar)
            load_k(3, nc.scalar)

            qx = {}

            def load_q(nch, eng):
                t = xq_pool.tile([P, nkt * 512], bf16, name="xq",
                                 tag=f"xq{nch % 2}")
                eng.dma_start(t.rearrange("p (a n) -> p a n", a=nkt),
                              _pmajor(qT)[:, :, nch * 512:(nch + 1) * 512])
                qx[nch] = t

            load_q(0, nc.sync)

            for c in range(nch_n):
                t = xv_pool.tile([P, nkt * 512], bf16, name="xv", tag=f"xv{c}")
                nc.gpsimd.dma_start(
                    t.rearrange("p (a n) -> p a n", a=nkt),
                    _pmajor(vT)[:, :, c * 512:(c + 1) * 512])
                vx[c] = t

            # ---- projection emit helpers ----------------------------------
            def proj_mm_steps(nch, xt_fn, wsl, dst, bias):
                """Matmul steps (~2 matmuls each) for one 512-col chunk.
                Shares one cell dict across steps; xt resolved lazily so the
                input-load step may precede the first matmul step."""
                cell = {}
                steps = []
                for f in range(nft):
                    for kt0 in range(0, nkt, 2):
                        def s_mm(f=f, kt0=kt0):
                            if kt0 == 0:
                                cell[f] = ps_misc.tile([P, 512], f32,
                                                       tag="ps", name="ps")
                            ps = cell[f]
                            xt = xt_fn()
                            c0 = nch * 512
                            for kt in (kt0, kt0 + 1):
                                nc.tensor.matmul(
                                    ps[:], lhsT=wsl(kt, f),
                                    rhs=xt[:, kt * 512:(kt + 1) * 512],
                                    start=(kt == 0), stop=(kt == nkt - 1))
                            if kt0 == nkt - 2:
                                if bias is not None:
                                    nc.vector.tensor_scalar_add(
                                        dst[f][:, c0:c0 + 512], ps[:],
                                        bias[:, f:f + 1])
                                else:
                                    nc.vector.tensor_copy(
                                        dst[f][:, c0:c0 + 512], ps[:])
                        steps.append(s_mm)
                return steps

            def q_proj_steps(nch):
                return ([lambda nch=nch: load_q(nch, nc.sync)]
                        + proj_mm_steps(nch, lambda nch=nch: qx[nch],
                                        wq_sl, QT_sb, bq_sb))

            # K projection emitted inline up front (scores need all of K^T);
            # Q chunk 0 inline after it. Later Q chunks are deferred.
            for nch in range(nch_n):
                for st in proj_mm_steps(nch, lambda nch=nch: kx[nch],
                                        wk_sl, KT_sb, None):
                    st()
            for st in proj_mm_steps(0, lambda: qx[0], wq_sl, QT_sb, bq_sb):
                st()

            # ---- V projection: [s, FLOC] with a ones column per head ------
            def v_proj_tile(tt):
                ps = ps_misc.tile([P, FLOC], f32, tag="ps", name="vps")
                xc = vx[tt // 4]
                o = (tt % 4) * P
                for kt in range(nkt):
                    nc.tensor.matmul(
                        ps[:], lhsT=xc[:, kt * 512 + o:kt * 512 + o + P],
                        rhs=wv_sl(kt),
                        start=(kt == 0), stop=(kt == nkt - 1))
                dst = V_sb[tt].rearrange("p (h x) -> p h x", x=DK + 1)
                nc.vector.tensor_copy(dst[:, :, 0:DK],
                                      ps.rearrange("p (h x) -> p h x", x=DK))
                nc.vector.memset(dst[:, :, DK:DK + 1], 1.0)

            # ---- output projection partial + ReduceScatter per half-chunk -
            def out_proj_steps(qc):
                steps = []
                for hc in range(2):
                    qcx = 2 * qc + hc
                    h0 = qc * qcw + hc * hw_
                    cell = {}
                    for od in range(nkt):
                        def s_mm(qcx=qcx, h0=h0, od=od, cell=cell):
                            if od == 0:
                                cell["rsb"] = rsb_pool.tile(
                                    [P, nkt * hw_], bf16, name="rsb")
                            ps = ps_misc.tile([P, hw_], f32, tag="ps",
                                              name="ops")
                            for ft in range(nft):
                                nc.tensor.matmul(
                                    ps[:], lhsT=wo_sl(ft, od),
                                    rhs=AOT_sb[ft][:, h0:h0 + hw_],
                                    start=(ft == 0), stop=(ft == nft - 1))
                            nc.vector.tensor_copy(
                                cell["rsb"][:, od * hw_:(od + 1) * hw_], ps[:])
                        steps.append(s_mm)

                    def s_ship(qcx=qcx, cell=cell):
                        nc.gpsimd.dma_start(
                            _pmajor(rs_in[qcx]),
                            cell["rsb"].rearrange("p (a n) -> p a n", a=nkt))
                        nc.gpsimd.collective_compute(
                            "ReduceScatter", mybir.AluOpType.add,
                            replica_groups=GROUPS,
                            ins=[rs_in[qcx]], outs=[rs_out[qcx]])
                        nc.gpsimd.dma_start(
                            outT[:, qcx * hw_:(qcx + 1) * hw_], rs_out[qcx])
                    steps.append(s_ship)
                return steps

            # ---- attention: software-pipelined scores -> exp -> attnV ------
            fifo = []      # pending attnV / normalize closures (1-3 it delay)
            slack = []     # deferred projection / output-proj steps

            def drain_slack():
                if slack:
                    slack.pop(0)()

            def drain_fifo(target):
                while len(fifo) > target:
                    fifo.pop(0)()

            for qc in range(4):
                q0 = qc * qcw
                if qc + 1 < nch_n:
                    slack.extend(q_proj_steps(qc + 1))
                if qc > 0:
                    slack.extend(out_proj_steps(qc - 1))
                for h in range(4):
                    ft, r0 = h // 2, (h % 2) * DK
                    hsl = slice(r0, r0 + DK)
                    havt = ps_acc.tile([DK + 1, qcw], f32)
                    for kg in range(nst // 2):
                        if qc == 0 and h == 0:
                            v_proj_tile(2 * kg)
                            v_proj_tile(2 * kg + 1)
                        sps = ps_s.tile([P, 2 * qcw], f32)
                        for j in range(2):
                            kt = kg * 2 + j
                            nc.tensor.matmul(
                                sps[:, j * qcw:(j + 1) * qcw],
                                lhsT=KT_sb[ft][hsl, kt * P:(kt + 1) * P],
                                rhs=QT_sb[ft][hsl, q0:q0 + qcw],
                                start=True, stop=True)
                        ex = exp_pool.tile([P, 2 * qcw], bf16)
                        nc.scalar.activation(ex[:], sps[:], EXP, scale=1.0 / 8.0)
                        if use_mask:
                            mt = msk_pool.tile([P, 2 * qcw], bf16)
                            nc.sync.dma_start(
                                mt.rearrange("p (a n) -> p a n", a=2),
                                _pmajor(maskT)[:, 2 * kg:2 * kg + 2,
                                               q0:q0 + qcw])
                            nc.vector.tensor_mul(ex[:], ex[:], mt[:])

                        def attnv(havt=havt, ex=ex, kg=kg, h=h):
                            for j in range(2):
                                kt = kg * 2 + j
                                nc.tensor.matmul(
                                    havt[:],
                                    lhsT=V_sb[kt][:, h * (DK + 1):
                                                  (h + 1) * (DK + 1)],
                                    rhs=ex[:, j * qcw:(j + 1) * qcw],
                                    start=(kt == 0), stop=(kt == nst - 1))
                        fifo.append(attnv)
                        drain_fifo(2)
                        drain_slack()

                    # per-head normalization, split in two pipeline entries so
                    # the bps matmul never waits on fresh DVE results
                    celln = {}

                    def norm1_step(havt=havt, celln=celln):
                        raw = small_pool.tile([DK, qcw], bf16, tag="raw",
                                              bufs=3, name="raw")
                        nc.vector.tensor_copy(raw[:], havt[0:DK, :])
                        den = small_pool.tile([1, qcw], f32, tag="den",
                                              bufs=2, name="den")
                        nc.vector.tensor_copy(den[:], havt[DK:DK + 1, :])
                        rec = small_pool.tile([1, qcw], f32, tag="rec",
                                              bufs=2, name="rec")
                        nc.vector.reciprocal_approx_fast(rec[:], den[:])
                        recb = small_pool.tile([1, qcw], bf16, tag="recb",
                                               bufs=2, name="recb")
                        nc.vector.tensor_copy(recb[:], rec[:])
                        celln["raw"], celln["recb"] = raw, recb

                    def norm2_step(ft=ft, hsl=hsl, q0=q0, celln=celln):
                        bps = ps_misc.tile([DK, qcw], f32, tag="ps",
                                           name="bps")
                        nc.tensor.matmul(bps[:], lhsT=ones_sb[:],
                                         rhs=celln["recb"][:],
                                         start=True, stop=True)
                        nc.vector.tensor_mul(
                            AOT_sb[ft][hsl, q0:q0 + qcw],
                            celln["raw"][:], bps[:])
                    fifo.append(norm1_step)
                    fifo.append(norm2_step)
                # chunk boundary: AOT(qc) must be complete before its
                # out_proj steps (drained during qc+1) read it
                drain_fifo(0)
            while slack:
                drain_slack()
            for st in out_proj_steps(3):
                st()

    nc.compile()
    _BUILD_CACHE[key] = nc
    return nc


def _in_maps(q, k, v, mask, Wq, bq, Wk, Wv, Wo, use_mask):
    maps = []
    maskT01 = None
    if use_mask:
        maskT01 = np.ascontiguousarray(
            (np.asarray(mask)[0, 0].T != 0)).astype(BF16)
    for c in range(N_CORES):
        b, g = c // 4, c % 4
        fs = slice(g * FLOC, (g + 1) * FLOC)
        m = {
            "qT": np.asarray(q[b]).T.astype(BF16),
            "kT": np.asarray(k[b]).T.astype(BF16),
            "vT": np.asarray(v[b]).T.astype(BF16),
            "wq": np.asarray(Wq)[:, fs].astype(BF16),
            "wk": np.asarray(Wk)[:, fs].astype(BF16),
            "wv": np.asarray(Wv)[:, fs].astype(BF16),
            "wo": np.ascontiguousarray(np.asarray(Wo)[fs, :]).astype(BF16),
            "bqp": np.asarray(bq)[fs].astype(np.float32).reshape(FLOC, 1),
        }
        if use_mask:
            m["maskT"] = maskT01
        maps.append(m)
    return maps


def kernel(q, k, v, mask, Wq, bq, Wk, bk, Wv, bv, Wo, bo):
    q, k, v = np.asarray(q), np.asarray(k), np.asarray(v)
    mask = np.asarray(mask)
    use_mask = not bool((mask != 0).all())
    nc = _build(S, use_mask)
    maps = _in_maps(q, k, v, mask, Wq, bq, Wk, Wv, Wo, use_mask)
    res = run_bass_kernel_spmd(nc, maps, list(range(N_CORES)), trace=TRACE)
    LAST["exec_time_ns"] = res.exec_time_ns
    LAST["results"] = res

    out = np.empty((B, S, D), np.float32)
    for c in range(N_CORES):
        b, g = c // 4, c % 4
        blk = np.asarray(res.results[c]["outT"]).astype(np.float32)
        out[b, :, g * FLOC:(g + 1) * FLOC] = blk.T
    # bk is a softmax no-op; bv rides through softmax (rows sum to 1) into
    # an effective output bias bv @ Wo + bo.
    bo_eff = (np.asarray(bv, np.float64) @ np.asarray(Wo, np.float64)
              + np.asarray(bo, np.float64)).astype(np.float32)
    out += bo_eff[None, None, :]
    return out


# revision 14
# speedup vs baseline: 1.1681x; 1.1681x over previous
"""Multi-head attention (B=2, S=2048, D=1024, H=16) on 8 Trainium2 NeuronCores.

Sharding: core c -> (batch b = c//4, head group g = c%4), i.e. data parallel on
batch and tensor parallel on heads (4 heads = 256 features per core) for the
QKV projections. Attention runs fully local per (batch, head-group). The
output projection is computed as a LOCAL partial product against the row-slice
Wo[g*256:(g+1)*256, :] (full 1024 output columns), and the 4 partials of a
group are combined with a ReduceScatter(add) per 256-token q-range whose
output (copied dram->dram) is the kernel's external output. This keeps every
matmul free of collective dependencies (the collective is a pure sink), unlike
an AllGather-then-project scheme where the projection matmuls stall the
in-order PE queue while the gather is in flight.

Scheduling notes (the in-order engine queues are the whole game):
  - The DMA fabric is a single ~250 GB/s resource shared by all engine
    queues round-robin, so inputs are issued on ONE queue (sync) in exact
    consumption order: wk, kT c0, wq, qT c0, wv, vT c0, (kT,vT) c1..c3, wo.
    Spreading them across queues delays the critical-path bytes.
  - The attention inner loop is software-pipelined: attnV(kg) is emitted 1-3
    iterations after exp(kg), so the ScalarEngine's exp latency (~1.1us per
    tile) never blocks the PE, which both removes stalls and keeps the PE
    p-state high (idle gaps drop it from 2.4 to 1.2 GHz).
  - Deferred work (K^T projection chunks 1-3, later Q^T chunks, the previous
    chunk's output projection, per-head softmax normalization) drains into
    the pipeline gaps a few steps per inner iteration.
  - The last 512-token q-chunk is processed as two 256-token mini-chunks so
    its first ReduceScatter overlaps the remaining attention; only the final
    RS (~13us) is an unavoidable tail.

Math notes (exact, not approximations):
  - bk is dropped: adding bk shifts every score in a row by a constant, and
    softmax is invariant to row-constant shifts.
  - bv and bo are folded into a single host-side bias add: softmax rows sum
    to 1, so attn @ (1 bv^T) = bv broadcast, and (out + bv) @ Wo + bo =
    out @ Wo + (bv @ Wo + bo).
  - bq is added on-device in the Q^T projection epilogue (per-partition add).
  - softmax skips max-subtraction: scores are ~N(0,1) for this problem's
    input distribution (|s| < ~7), far from fp32/bf16 exp overflow.
  - an all-ones mask (this problem's spec) is an identity; if a mask with
    zeros is ever passed, a masked kernel variant is compiled instead
    (multiply exp(scores) by the 0/1 mask — identical to adding -1e9).

Compute is bf16 on the TensorEngine (fp32 PSUM accumulation), exp on the
ScalarEngine in fp32. Scores are computed transposed (S^T[k_tok, q]) so that
attn @ V needs no transposes and the softmax denominator is obtained free via
an extra ones-column appended to V.
"""

import numpy as np
import ml_dtypes

try:
    import concourse.bass as bass  # noqa: F401
except ImportError:  # fresh interpreter without the repo on sys.path
    import sys

    for p in ("/opt/trn_rl_repo", "/root/.axon_site/_ro/trn_rl_repo"):
        if p not in sys.path:
            sys.path.insert(0, p)
    import concourse.bass as bass  # noqa: F401

import concourse.tile as tile
from concourse import bacc, mybir
from concourse.bass_utils import run_bass_kernel_spmd

BF16 = ml_dtypes.bfloat16
B, S, D, H = 2, 2048, 1024, 16
DK = D // H            # 64
N_CORES = 8
GROUPS = [[0, 1, 2, 3], [4, 5, 6, 7]]
FLOC = D // 4          # 256 features (4 heads) per core
P = 128

TRACE = False
LAST = {}

_BUILD_CACHE = {}


def _pmajor(ap):
    """View a [A*128, N] DRAM tensor as [128, A, N] (partition-major)."""
    return ap.rearrange("(a p) n -> p a n", p=P)


def _build(s, use_mask):
    key = (s, use_mask)
    if key in _BUILD_CACHE:
        return _BUILD_CACHE[key]

    f32 = mybir.dt.float32
    bf16 = mybir.dt.bfloat16
    nkt = D // P               # 8 k-tiles over the model dim
    nst = s // P               # 16 seq tiles of 128
    nft = FLOC // P            # 2 feature tiles per core
    nch_n = s // 512           # 4 input chunks for the projections
    hw_ = 256                  # ReduceScatter q-range width
    # attention chunks (q0, width); the last 512 split into two 256s so the
    # final collective overlaps compute
    AC = [(0, 512), (512, 512), (1024, 512), (1536, 256), (1792, 256)]
    n_rs = s // hw_            # 8 ReduceScatter ops

    nc = bacc.Bacc("TRN2", target_bir_lowering=False, debug=False,
                   enable_asserts=True, num_devices=N_CORES)

    qT = nc.dram_tensor("qT", [D, s], bf16, kind="ExternalInput").ap()
    kT = nc.dram_tensor("kT", [D, s], bf16, kind="ExternalInput").ap()
    vT = nc.dram_tensor("vT", [D, s], bf16, kind="ExternalInput").ap()
    wq = nc.dram_tensor("wq", [D, FLOC], bf16, kind="ExternalInput").ap()
    wk = nc.dram_tensor("wk", [D, FLOC], bf16, kind="ExternalInput").ap()
    wv = nc.dram_tensor("wv", [D, FLOC], bf16, kind="ExternalInput").ap()
    # Row-slice of Wo: Wo[g*FLOC:(g+1)*FLOC, :] — this core's contraction rows
    wo = nc.dram_tensor("wo", [FLOC, D], bf16, kind="ExternalInput").ap()
    bqp = nc.dram_tensor("bqp", [FLOC, 1], f32, kind="ExternalInput").ap()
    if use_mask:
        maskT = nc.dram_tensor("maskT", [s, s], bf16, kind="ExternalInput").ap()

    rs_in = [nc.dram_tensor(f"rs_in{x}", [D, hw_], bf16).ap()
             for x in range(n_rs)]
    rs_out = [nc.dram_tensor(f"rs_out{x}", [FLOC, hw_], bf16).ap()
              for x in range(n_rs)]
    # outT[outd, q] — columns x*hw_.. filled by a dram->dram copy after RS x
    outT = nc.dram_tensor("outT", [FLOC, s], bf16, kind="ExternalOutput").ap()

    EXP = mybir.ActivationFunctionType.Exp

    with tile.TileContext(nc) as tc:
        with (
            tc.tile_pool(name="persist", bufs=1) as pp,
            tc.tile_pool(name="xq", bufs=2) as xq_pool,
            tc.tile_pool(name="xk", bufs=1) as xk_pool,
            tc.tile_pool(name="xv", bufs=1) as xv_pool,
            tc.tile_pool(name="exp", bufs=5) as exp_pool,
            tc.tile_pool(name="msk", bufs=4) as msk_pool,
            tc.tile_pool(name="small", bufs=4) as small_pool,
            tc.tile_pool(name="rsb", bufs=1) as rsb_pool,
            tc.tile_pool(name="ps_s", bufs=2, space="PSUM") as ps_s,
            tc.tile_pool(name="ps_acc", bufs=2, space="PSUM") as ps_acc,
            tc.tile_pool(name="ps_misc", bufs=2, space="PSUM") as ps_misc,
        ):
            w_sb = {}

            def load_w(nm, src, a):
                t = pp.tile([P, a * src.shape[1]], bf16, tag=nm, name=nm)
                nc.sync.dma_start(t.rearrange("p (a n) -> p a n", a=a),
                                  _pmajor(src))
                w_sb[nm] = t

            wq_sl = lambda kt, f: w_sb["wq"][:, kt * FLOC + f * P: kt * FLOC + (f + 1) * P]
            wk_sl = lambda kt, f: w_sb["wk"][:, kt * FLOC + f * P: kt * FLOC + (f + 1) * P]
            wv_sl = lambda kt: w_sb["wv"][:, kt * FLOC:(kt + 1) * FLOC]
            wo_sl = lambda ft, od: w_sb["wo"][:, ft * D + od * P: ft * D + (od + 1) * P]

            QT_sb = [pp.tile([P, s], bf16, tag=f"qtsb{f}", name=f"qtsb{f}")
                     for f in range(nft)]
            KT_sb = [pp.tile([P, s], bf16, tag=f"ktsb{f}", name=f"ktsb{f}")
                     for f in range(nft)]
            AOT_sb = [pp.tile([P, s], bf16, tag=f"aot{f}", name=f"aot{f}")
                      for f in range(nft)]
            V_sb = [pp.tile([P, 4 * (DK + 1)], bf16, tag=f"vsb{tt}", name=f"vsb{tt}")
                    for tt in range(nst)]

            kx, vx, qx = {}, {}, {}

            def load_x(dst, pool, src, nch, tag):
                t = pool.tile([P, nkt * 512], bf16, name=tag, tag=tag)
                nc.sync.dma_start(t.rearrange("p (a n) -> p a n", a=nkt),
                                  _pmajor(src)[:, :, nch * 512:(nch + 1) * 512])
                dst[nch] = t

            # ---- startup DMA stream: one queue, consumption order ---------
            load_w("wk", wk, nkt)
            load_x(kx, xk_pool, kT, 0, "xk0")
            load_w("wq", wq, nkt)
            bq_sb = pp.tile([P, nft], f32, tag="bq", name="bq")
            nc.sync.dma_start(
                bq_sb.rearrange("p (a n) -> p a n", a=nft), _pmajor(bqp))
            load_x(qx, xq_pool, qT, 0, "xq0")
            load_w("wv", wv, nkt)
            load_x(vx, xv_pool, vT, 0, "xv0")
            for c in range(1, nch_n):
                load_x(kx, xk_pool, kT, c, f"xk{c}")
                load_x(vx, xv_pool, vT, c, f"xv{c}")
            load_w("wo", wo, nft)   # [128, nft*D]

            ones_sb = pp.tile([1, DK], bf16, tag="ones", name="ones")
            nc.vector.memset(ones_sb[:], 1.0)

            # ---- projection helpers ---------------------------------------
            def proj_mm_steps(nch, xt_fn, wsl, dst, bias):
                cell = {}
                steps = []
                for f in range(nft):
                    for kt0 in range(0, nkt, 2):
                        def s_mm(f=f, kt0=kt0):
                            if kt0 == 0:
                                cell[f] = ps_misc.tile([P, 512], f32,
                                                       tag="ps", name="ps")
                            ps = cell[f]
                            xt = xt_fn()
                            c0 = nch * 512
                            for kt in (kt0, kt0 + 1):
                                nc.tensor.matmul(
                                    ps[:], lhsT=wsl(kt, f),
                                    rhs=xt[:, kt * 512:(kt + 1) * 512],
                                    start=(kt == 0), stop=(kt == nkt - 1))
                            if kt0 == nkt - 2:
                                if bias is not None:
                                    nc.vector.tensor_scalar_add(
                                        dst[f][:, c0:c0 + 512], ps[:],
                                        bias[:, f:f + 1])
                                else:
                                    nc.vector.tensor_copy(
                                        dst[f][:, c0:c0 + 512], ps[:])
                        steps.append(s_mm)
                return steps

            def q_proj_steps(nch):
                return ([lambda nch=nch: load_x(qx, xq_pool, qT, nch,
                                                f"xq{nch % 2}")]
                        + proj_mm_steps(nch, lambda nch=nch: qx[nch],
                                        wq_sl, QT_sb, bq_sb))

            # K chunk 0 + Q chunk 0 inline; K chunks 1-3 deferred to slack
            for st in proj_mm_steps(0, lambda: kx[0], wk_sl, KT_sb, None):
                st()
            for st in proj_mm_steps(0, lambda: qx[0], wq_sl, QT_sb, bq_sb):
                st()
            k_slack = []
            for nch in range(1, nch_n):
                k_slack += proj_mm_steps(nch, lambda nch=nch: kx[nch],
                                         wk_sl, KT_sb, None)

            def v_proj_tile(tt):
                ps = ps_misc.tile([P, FLOC], f32, tag="ps", name="vps")
                xc = vx[tt // 4]
                o = (tt % 4) * P
                for kt in range(nkt):
                    nc.tensor.matmul(
                        ps[:], lhsT=xc[:, kt * 512 + o:kt * 512 + o + P],
                        rhs=wv_sl(kt),
                        start=(kt == 0), stop=(kt == nkt - 1))
                dst = V_sb[tt].rearrange("p (h x) -> p h x", x=DK + 1)
                nc.vector.tensor_copy(dst[:, :, 0:DK],
                                      ps.rearrange("p (h x) -> p h x", x=DK))
                nc.vector.memset(dst[:, :, DK:DK + 1], 1.0)

            # ---- output projection partial + ReduceScatter ----------------
            def out_proj_steps(ci):
                q0, w = AC[ci]
                steps = []
                cell = {}
                for od in range(nkt):
                    def s_mm(q0=q0, w=w, od=od, cell=cell):
                        if od == 0:
                            cell["rsb"] = rsb_pool.tile(
                                [P, nkt * w], bf16, name="rsb")
                        ps = ps_misc.tile([P, w], f32, tag="ps", name="ops")
                        for ft in range(nft):
                            nc.tensor.matmul(
                                ps[:], lhsT=wo_sl(ft, od),
                                rhs=AOT_sb[ft][:, q0:q0 + w],
                                start=(ft == 0), stop=(ft == nft - 1))
                        nc.vector.tensor_copy(
                            cell["rsb"][:, od * w:(od + 1) * w], ps[:])
                    steps.append(s_mm)
                for hc in range(w // hw_):
                    def s_ship(q0=q0, w=w, hc=hc, cell=cell):
                        qcx = (q0 + hc * hw_) // hw_
                        rsb = cell["rsb"].rearrange("p (a n) -> p a n", a=nkt)
                        nc.gpsimd.dma_start(
                            _pmajor(rs_in[qcx]),
                            rsb[:, :, hc * hw_:(hc + 1) * hw_])
                        nc.gpsimd.collective_compute(
                            "ReduceScatter", mybir.AluOpType.add,
                            replica_groups=GROUPS,
                            ins=[rs_in[qcx]], outs=[rs_out[qcx]])
                        nc.gpsimd.dma_start(
                            outT[:, qcx * hw_:(qcx + 1) * hw_], rs_out[qcx])
                    steps.append(s_ship)
                return steps

            # ---- attention: software-pipelined scores -> exp -> attnV ------
            fifo = []      # pending attnV / normalize closures
            slack = []     # deferred projection / output-projection steps

            def drain_slack(n=1):
                for _ in range(n):
                    if k_slack:
                        k_slack.pop(0)()
                    elif slack:
                        slack.pop(0)()

            def drain_fifo(target):
                while len(fifo) > target:
                    fifo.pop(0)()

            q_emitted = {0}
            for ci, (q0, w) in enumerate(AC):
                if ci + 1 < len(AC):
                    nxt = AC[ci + 1][0] // 512
                    if nxt not in q_emitted:
                        q_emitted.add(nxt)
                        slack.extend(q_proj_steps(nxt))
                if ci > 0:
                    slack.extend(out_proj_steps(ci - 1))
                for h in range(4):
                    ft, r0 = h // 2, (h % 2) * DK
                    hsl = slice(r0, r0 + DK)
                    havt = ps_acc.tile([DK + 1, w], f32)
                    for kg in range(nst // 2):
                        sps = ps_s.tile([P, 2 * w], f32)
                        for j in range(2):
                            kt = kg * 2 + j
                            nc.tensor.matmul(
                                sps[:, j * w:(j + 1) * w],
                                lhsT=KT_sb[ft][hsl, kt * P:(kt + 1) * P],
                                rhs=QT_sb[ft][hsl, q0:q0 + w],
                                start=True, stop=True)
                        ex = exp_pool.tile([P, 2 * w], bf16)
                        nc.scalar.activation(ex[:], sps[:], EXP, scale=1.0 / 8.0)
                        if use_mask:
                            mt = msk_pool.tile([P, 2 * w], bf16)
                            nc.sync.dma_start(
                                mt.rearrange("p (a n) -> p a n", a=2),
                                _pmajor(maskT)[:, 2 * kg:2 * kg + 2,
                                               q0:q0 + w])
                            nc.vector.tensor_mul(ex[:], ex[:], mt[:])
                        if ci == 0 and h == 0:
                            v_proj_tile(2 * kg)
                            v_proj_tile(2 * kg + 1)

                        def attnv(havt=havt, ex=ex, kg=kg, h=h, w=w):
                            for j in range(2):
                                kt = kg * 2 + j
                                nc.tensor.matmul(
                                    havt[:],
                                    lhsT=V_sb[kt][:, h * (DK + 1):
                                                  (h + 1) * (DK + 1)],
                                    rhs=ex[:, j * w:(j + 1) * w],
                                    start=(kt == 0), stop=(kt == nst - 1))
                        fifo.append(attnv)
                        drain_fifo(2)
                        drain_slack(4 if (ci == 0 and h == 0) else 1)

                    # per-head normalization, split in two pipeline entries so
                    # the bps matmul never waits on fresh DVE results
                    celln = {}

                    def norm1_step(havt=havt, w=w, celln=celln):
                        raw = small_pool.tile([DK, w], bf16, tag="raw",
                                              bufs=3, name="raw")
                        nc.vector.tensor_copy(raw[:], havt[0:DK, :])
                        den = small_pool.tile([1, w], f32, tag="den",
                                              bufs=2, name="den")
                        nc.vector.tensor_copy(den[:], havt[DK:DK + 1, :])
                        rec = small_pool.tile([1, w], f32, tag="rec",
                                              bufs=2, name="rec")
                        nc.vector.reciprocal_approx_fast(rec[:], den[:])
                        recb = small_pool.tile([1, w], bf16, tag="recb",
                                               bufs=2, name="recb")
                        nc.vector.tensor_copy(recb[:], rec[:])
                        celln["raw"], celln["recb"] = raw, recb

                    def norm2_step(ft=ft, hsl=hsl, q0=q0, w=w, celln=celln):
                        bps = ps_misc.tile([DK, w], f32, tag="ps",
                                           name="bps")
                        nc.tensor.matmul(bps[:], lhsT=ones_sb[:],
                                         rhs=celln["recb"][:],
                                         start=True, stop=True)
                        nc.vector.tensor_mul(
                            AOT_sb[ft][hsl, q0:q0 + w],
                            celln["raw"][:], bps[:])
                    fifo.append(norm1_step)
                    fifo.append(norm2_step)
                # chunk boundary: AOT(ci) must be complete before its
                # out_proj steps (drained during ci+1) read it
                drain_fifo(0)
            while k_slack or slack:
                drain_slack()
            for st in out_proj_steps(len(AC) - 1):
                st()

    nc.compile()
    _BUILD_CACHE[key] = nc
    return nc


def _in_maps(q, k, v, mask, Wq, bq, Wk, Wv, Wo, use_mask):
    maps = []
    maskT01 = None
    if use_mask:
        maskT01 = np.ascontiguousarray(
            (np.asarray(mask)[0, 0].T != 0)).astype(BF16)
    for c in range(N_CORES):
        b, g = c // 4, c % 4
        fs = slice(g * FLOC, (g + 1) * FLOC)
        m = {
            "qT": np.asarray(q[b]).T.astype(BF16),
            "kT": np.asarray(k[b]).T.astype(BF16),
            "vT": np.asarray(v[b]).T.astype(BF16),
            "wq": np.asarray(Wq)[:, fs].astype(BF16),
            "wk": np.asarray(Wk)[:, fs].astype(BF16),
            "wv": np.asarray(Wv)[:, fs].astype(BF16),
            "wo": np.ascontiguousarray(np.asarray(Wo)[fs, :]).astype(BF16),
            "bqp": np.asarray(bq)[fs].astype(np.float32).reshape(FLOC, 1),
        }
        if use_mask:
            m["maskT"] = maskT01
        maps.append(m)
    return maps


def kernel(q, k, v, mask, Wq, bq, Wk, bk, Wv, bv, Wo, bo):
    q, k, v = np.asarray(q), np.asarray(k), np.asarray(v)
    mask = np.asarray(mask)
    use_mask = not bool((mask != 0).all())
    nc = _build(S, use_mask)
    maps = _in_maps(q, k, v, mask, Wq, bq, Wk, Wv, Wo, use_mask)
    res = run_bass_kernel_spmd(nc, maps, list(range(N_CORES)), trace=TRACE)
    LAST["exec_time_ns"] = res.exec_time_ns
    LAST["results"] = res

    out = np.empty((B, S, D), np.float32)
    for c in range(N_CORES):
        b, g = c // 4, c % 4
        blk = np.asarray(res.results[c]["outT"]).astype(np.float32)
        out[b, :, g * FLOC:(g + 1) * FLOC] = blk.T
    # bk is a softmax no-op; bv rides through softmax (rows sum to 1) into
    # an effective output bias bv @ Wo + bo.
    bo_eff = (np.asarray(bv, np.float64) @ np.asarray(Wo, np.float64)
              + np.asarray(bo, np.float64)).astype(np.float32)
    out += bo_eff[None, None, :]
    return out
